# revision 4
# baseline (speedup 1.0000x reference)
"""NoisyHadamardLinear Trainium2 kernel (self-contained).

y = blockwise_FHT_1024(x) @ W^T + b  for x [2, 4096, 4096], W [4096, 4096],
b [4096], computed on 8 NeuronCores, data-parallel over the 8192 tokens
(1024 tokens per core).

Per-core pipeline (all matmuls fp32r on TensorE):
  phase H: PE-transpose x tiles -> xT [d, t]; Hadamard applied as matmuls
           against +-H_128/32 (H_1024 = H_8 (x) H_128 Kronecker; the H_8 sign
           is compile-time) accumulating in PSUM -> xhT tiles resident in SBUF.
  phase M: per 256-wide o-slab, PE-transpose W tiles on the fly -> WT;
           y[t, o] = sum_d xhT[d, t].T @ WT[d, o] accumulated over 32 d-tiles
           in PSUM, bias added as a rank-1 (ones x b) matmul; ACT evict; DMA.
"""
import numpy as np

import concourse.bacc as bacc
import concourse.mybir as mybir
import concourse.tile as tile
from concourse.bass_utils import run_bass_kernel_spmd

P = 128
f32r = mybir.dt.float32r
f32 = mybir.dt.float32

N_CORES = 8
B, S, D, O = 2, 4096, 4096, 4096
T = (B * S) // N_CORES  # tokens per core


def _h8_sign(u, v):
    return 1 if bin(u & v).count("1") % 2 == 0 else -1


def _hadamard128():
    h = np.array([[1.0]], dtype=np.float32)
    while h.shape[0] < P:
        h = np.block([[h, h], [h, -h]])
    return h.astype(np.float32)


def build_kernel(T=T, D=D, O=O, OS=512, num_devices=N_CORES):
    """T tokens per core; D in-features; O out-features; OS o-slab width."""
    NTH = T // 512 if T >= 512 else 1      # t-halves (512 tokens each)
    TH = T // NTH                           # tokens per half
    NTS = TH // P                           # t-subtiles per half
    NBLK = D // 1024                        # hadamard blocks
    ND = D // P                             # d tiles
    NOS = O // OS                           # o-slabs
    NOSUB = OS // P                         # o-subtiles per slab
    NDCH = D // 1024                        # d-chunks for W loads

    nc = bacc.Bacc("TRN2", target_bir_lowering=False, debug=False,
                   num_devices=num_devices, dynamic_dma_scratch_size=2048)
    x = nc.dram_tensor("x", [T, D], f32r, kind="ExternalInput")
    W = nc.dram_tensor("W", [O, D], f32r, kind="ExternalInput")
    b = nc.dram_tensor("b", [1, O], f32r, kind="ExternalInput")
    Hp = nc.dram_tensor("Hp", [P, P], f32r, kind="ExternalInput")
    Hn = nc.dram_tensor("Hn", [P, P], f32r, kind="ExternalInput")
    Ident = nc.dram_tensor("Ident", [P, P], f32r, kind="ExternalInput")
    Ones = nc.dram_tensor("Ones", [1, P], f32r, kind="ExternalInput")
    y = nc.dram_tensor("y", [T, O], f32, kind="ExternalOutput")

    with tile.TileContext(nc) as tc:
        with tc.tile_pool(name="const", bufs=1) as cpool, \
             tc.tile_pool(name="xhT", bufs=ND) as xhTp, \
             tc.tile_pool(name="tps", bufs=3, space="PSUM") as tps, \
             tc.tile_pool(name="hps", bufs=2, space="PSUM") as hps, \
             tc.tile_pool(name="yps", bufs=3, space="PSUM") as yps:
            ident = cpool.tile([P, P], f32r)
            hp = cpool.tile([P, P], f32r)
            hn = cpool.tile([P, P], f32r)
            ones = cpool.tile([1, P], f32r)
            nc.sync.dma_start(ident[:], Ident.ap())
            nc.sync.dma_start(hp[:], Hp.ap())
            nc.sync.dma_start(hn[:], Hn.ap())
            nc.sync.dma_start(ones[:], Ones.ap())

            # persistent xhT tiles [128 d, T tokens]
            xhT = [xhTp.tile([P, T], f32r, tag="xhT", name=f"xhT{i}") for i in range(ND)]

            # ---------------- phase H ----------------
            with tc.tile_pool(name="xnat", bufs=NTS + 1) as xnat, \
                 tc.tile_pool(name="xTp", bufs=9) as xTp:
                for th in range(NTH):
                    for blk in range(NBLK):
                        xns = []
                        for ts in range(NTS):
                            xn = xnat.tile([P, 1024], f32r, tag="xn")
                            trow = (th * NTS + ts) * P
                            nc.sync.dma_start(
                                xn[:], x.ap()[trow:trow + P,
                                              blk * 1024:(blk + 1) * 1024])
                            xns.append(xn)
                        xTs = []
                        for u in range(8):
                            tp = tps.tile([P, TH], f32r, tag="tps")
                            for ts in range(NTS):
                                nc.tensor.transpose(
                                    tp[:, ts * P:(ts + 1) * P],
                                    xns[ts][:, u * P:(u + 1) * P], ident[:])
                            t = xTp.tile([P, TH], f32r, tag="xT")
                            nc.scalar.copy(t[:], tp[:])
                            xTs.append(t)
                        for v in range(8):
                            ph = hps.tile([P, TH], f32, tag="hps")
                            for u in range(8):
                                h = hp if _h8_sign(u, v) > 0 else hn
                                nc.tensor.matmul(ph[:], h[:], xTs[u][:],
                                                 start=(u == 0), stop=(u == 7))
                            nc.scalar.copy(
                                xhT[blk * 8 + v][:, th * TH:(th + 1) * TH],
                                ph[:])

            # ---------------- phase M ----------------
            NWCH = D // 512                # W-load d-chunks of 512
            with tc.tile_pool(name="wnat", bufs=NOSUB + 1) as wnat, \
                 tc.tile_pool(name="WTp", bufs=ND + 2) as WTp, \
                 tc.tile_pool(name="bpool", bufs=2) as bpool, \
                 tc.tile_pool(name="yout", bufs=2) as yout:
                for os_ in range(NOS):
                    bt = bpool.tile([1, OS], f32r, tag="bt")
                    nc.sync.dma_start(bt[:], b.ap()[:, os_ * OS:(os_ + 1) * OS])
                    WTs = []
                    for dch in range(NWCH):
                        wns = []
                        for osub in range(NOSUB):
                            wn = wnat.tile([P, 512], f32r, tag="wn")
                            orow = os_ * OS + osub * P
                            nc.sync.dma_start(
                                wn[:], W.ap()[orow:orow + P,
                                              dch * 512:(dch + 1) * 512])
                            wns.append(wn)
                        for dt in range(4):
                            tp = tps.tile([P, OS], f32r, tag="tps")
                            for osub in range(NOSUB):
                                nc.tensor.transpose(
                                    tp[:, osub * P:(osub + 1) * P],
                                    wns[osub][:, dt * P:(dt + 1) * P], ident[:])
                            t = WTp.tile([P, OS], f32r, tag="WT")
                            nc.vector.tensor_copy(t[:], tp[:])
                            WTs.append(t)
                    for ts in range(NTH * NTS):
                        py = yps.tile([P, OS], f32, tag="yps")
                        nc.tensor.matmul(py[:], ones[:1, :], bt[:1, :],
                                         start=True, stop=False)
                        for d in range(ND):
                            nc.tensor.matmul(py[:], xhT[d][:, ts * P:(ts + 1) * P],
                                             WTs[d][:],
                                             start=False, stop=(d == ND - 1))
                        yo = yout.tile([P, OS], f32, tag="yo")
                        nc.scalar.copy(yo[:], py[:])
                        nc.sync.dma_start(
                            y.ap()[ts * P:(ts + 1) * P,
                                   os_ * OS:(os_ + 1) * OS], yo[:])
    nc.compile()
    return nc


_CACHED_NC = None


def _get_nc():
    global _CACHED_NC
    if _CACHED_NC is None:
        _CACHED_NC = build_kernel()
    return _CACHED_NC


def kernel(x, W, b):
    x = np.asarray(x, dtype=np.float32)
    W = np.asarray(W, dtype=np.float32)
    b = np.asarray(b, dtype=np.float32)
    assert x.shape == (B, S, D) and W.shape == (O, D) and b.shape == (O,)

    nc = _get_nc()
    h128 = _hadamard128()
    consts = {
        "Hp": (h128 / 32.0).astype(np.float32),
        "Hn": (-h128 / 32.0).astype(np.float32),
        "Ident": np.eye(P, dtype=np.float32),
        "Ones": np.ones((1, P), np.float32),
    }
    xf = np.ascontiguousarray(x.reshape(B * S, D))
    in_maps = []
    for c in range(N_CORES):
        in_maps.append({
            "x": np.ascontiguousarray(xf[c * T:(c + 1) * T]),
            "W": W,
            "b": np.ascontiguousarray(b.reshape(1, O)),
            **consts,
        })
    res = run_bass_kernel_spmd(nc, in_maps, core_ids=list(range(N_CORES)))
    y = np.concatenate([res.results[c]["y"] for c in range(N_CORES)], axis=0)
    return y.reshape(B, S, O).astype(np.float32, copy=False)


# revision 5
# speedup vs baseline: 1.0388x; 1.0388x over previous
"""NoisyHadamardLinear Trainium2 kernel (self-contained).

y = blockwise_FHT_1024(x) @ W^T + b  for x [2, 4096, 4096], W [4096, 4096],
b [4096], on 8 NeuronCores, data-parallel over the 8192 tokens (1024/core).

Per-core pipeline (all matmuls fp32r on TensorE):
  phase H: PE-transpose x tiles -> xT chunks; apply H_128/32 as one matmul
           per 128-chunk with butterfly stage-1 folded into the PSUM
           accumulation (H_1024 = H_8 (x) H_128 Kronecker); butterfly
           stages 2-3 on VectorE -> xhT tiles [d, t] resident in SBUF.
  phase M: per 512-wide o-slab, PE-transpose W tiles on the fly -> WT;
           y[t, o] = sum_d xhT[d, t].T @ WT[d, o] accumulated over 32
           d-tiles in PSUM + bias rank-1 (ones x b) matmul; ACT evict; DMA.
"""
import numpy as np

import concourse.bacc as bacc
import concourse.mybir as mybir
import concourse.tile as tile
from concourse.bass_utils import run_bass_kernel_spmd

P = 128
f32r = mybir.dt.float32r
f32 = mybir.dt.float32

N_CORES = 8
B, S, D, O = 2, 4096, 4096, 4096
T_PER_CORE = (B * S) // N_CORES


def build_kernel(T=T_PER_CORE, D=D, O=O, OS=512, num_devices=N_CORES,
                 phases=('H', 'M')):
    NTH = 2 if T >= 1024 else 1            # t-halves
    TH = T // NTH                          # tokens per half
    NTS = TH // P                          # t-subtiles per half
    NBLK = D // 1024                       # hadamard blocks
    ND = D // P                            # d tiles
    NOS = O // OS                          # o-slabs
    NOSUB = OS // P                        # o-subtiles per slab

    nc = bacc.Bacc("TRN2", target_bir_lowering=False, debug=False,
                   num_devices=num_devices, dynamic_dma_scratch_size=2048)
    x = nc.dram_tensor("x", [T, D], f32r, kind="ExternalInput")
    W = nc.dram_tensor("W", [O, D], f32r, kind="ExternalInput")
    b = nc.dram_tensor("b", [1, O], f32r, kind="ExternalInput")
    Hp = nc.dram_tensor("Hp", [P, P], f32r, kind="ExternalInput")
    Hn = nc.dram_tensor("Hn", [P, P], f32r, kind="ExternalInput")
    Ident = nc.dram_tensor("Ident", [P, P], f32r, kind="ExternalInput")
    Ones = nc.dram_tensor("Ones", [1, P], f32r, kind="ExternalInput")
    y = nc.dram_tensor("y", [T, O], f32, kind="ExternalOutput")

    with tile.TileContext(nc) as tc:
        with tc.tile_pool(name="const", bufs=1) as cpool, \
             tc.tile_pool(name="xhT", bufs=ND) as xhTp:
            ident = cpool.tile([P, P], f32r)
            hp = cpool.tile([P, P], f32r)
            hn = cpool.tile([P, P], f32r)
            ones = cpool.tile([1, P], f32r)
            nc.sync.dma_start(ident[:], Ident.ap())
            nc.sync.dma_start(hp[:], Hp.ap())
            nc.sync.dma_start(hn[:], Hn.ap())
            nc.sync.dma_start(ones[:], Ones.ap())

            # persistent xhT tiles [128 d, T tokens]
            xhT = [xhTp.tile([P, T], f32r, tag="xhT", name=f"xhT{i}")
                   for i in range(ND)]

            if 'H' in phases:
                _phase_h(nc, tc, x, ident, hp, hn, xhT,
                         NTH, TH, NTS, NBLK)
            if 'M' in phases:
                _phase_m(nc, tc, W, b, ident, ones, xhT, y,
                         NTH, NTS, ND, NOS, NOSUB, OS, D)
    nc.compile()
    return nc


def _phase_h(nc, tc, x, ident, hp, hn, xhT, NTH, TH, NTS, NBLK):
    with tc.tile_pool(name="xnat", bufs=NTS + 1) as xnat, \
         tc.tile_pool(name="xTp", bufs=9) as xTp, \
         tc.tile_pool(name="bfp", bufs=20) as bfp, \
         tc.tile_pool(name="tps", bufs=3, space="PSUM") as tps, \
         tc.tile_pool(name="hps", bufs=3, space="PSUM") as hps:
        for th in range(NTH):
            for blk in range(NBLK):
                xns = []
                for ts in range(NTS):
                    xn = xnat.tile([P, 1024], f32r, tag="xn")
                    trow = (th * NTS + ts) * P
                    nc.sync.dma_start(
                        xn[:], x.ap()[trow:trow + P,
                                      blk * 1024:(blk + 1) * 1024])
                    xns.append(xn)
                # transpose x tiles -> xT chunks
                xTs = []
                for u in range(8):
                    tp = tps.tile([P, TH], f32r, tag="tps")
                    for ts in range(NTS):
                        nc.tensor.transpose(
                            tp[:, ts * P:(ts + 1) * P],
                            xns[ts][:, u * P:(u + 1) * P], ident[:])
                    t = xTp.tile([P, TH], f32r, tag="xT")
                    nc.scalar.copy(t[:], tp[:])
                    xTs.append(t)
                # H128/32 chunk matmuls with butterfly stage-1 folded into
                # PSUM accumulation: s_k = H(x_2k)+H(x_2k+1),
                # d_k = H(x_2k)-H(x_2k+1) (via -H on the second operand)
                cur = []
                for k in range(4):
                    for sign in range(2):
                        ph = hps.tile([P, TH], f32, tag="hps")
                        nc.tensor.matmul(ph[:], hp[:], xTs[2 * k][:],
                                         start=True, stop=False)
                        nc.tensor.matmul(ph[:],
                                         (hp if sign == 0 else hn)[:],
                                         xTs[2 * k + 1][:],
                                         start=False, stop=True)
                        z = bfp.tile([P, TH], f32r, tag="bf",
                                     name=f"z{th}_{blk}_{k}_{sign}")
                        nc.scalar.copy(z[:], ph[:])
                        cur.append(z)
                # H8 butterfly stages 2-3 on VectorE
                for s in range(1, 3):
                    stride = 1 << s
                    nxt = [bfp.tile([P, TH], f32r, tag="bf",
                                    name=f"bf{th}_{blk}_{s}_{v}")
                           if s < 2 else None
                           for v in range(8)]
                    for g in range(0, 8, 2 * stride):
                        for j in range(stride):
                            a = cur[g + j]
                            bb = cur[g + j + stride]
                            if s == 2:
                                oa = xhT[blk * 8 + g + j][
                                    :, th * TH:(th + 1) * TH]
                                ob = xhT[blk * 8 + g + j + stride][
                                    :, th * TH:(th + 1) * TH]
                            else:
                                oa = nxt[g + j][:]
                                ob = nxt[g + j + stride][:]
                            nc.vector.tensor_add(oa, a[:], bb[:])
                            nc.vector.tensor_sub(ob, a[:], bb[:])
                    cur = nxt


def _phase_m(nc, tc, W, b, ident, ones, xhT, y,
             NTH, NTS, ND, NOS, NOSUB, OS, D):
    NWCH = D // 512
    with tc.tile_pool(name="wnat", bufs=NOSUB + 1) as wnat, \
         tc.tile_pool(name="WTp", bufs=ND + 2) as WTp, \
         tc.tile_pool(name="bpool", bufs=2) as bpool, \
         tc.tile_pool(name="yout", bufs=2) as yout, \
         tc.tile_pool(name="tps", bufs=3, space="PSUM") as tps, \
         tc.tile_pool(name="yps", bufs=4, space="PSUM") as yps:
        for os_ in range(NOS):
            bt = bpool.tile([1, OS], f32r, tag="bt")
            nc.sync.dma_start(bt[:], b.ap()[:, os_ * OS:(os_ + 1) * OS])
            WTs = []
            for dch in range(NWCH):
                wns = []
                for osub in range(NOSUB):
                    wn = wnat.tile([P, 512], f32r, tag="wn")
                    orow = os_ * OS + osub * P
                    nc.sync.dma_start(
                        wn[:], W.ap()[orow:orow + P,
                                      dch * 512:(dch + 1) * 512])
                    wns.append(wn)
                for dt in range(4):
                    tp = tps.tile([P, OS], f32r, tag="tps")
                    for osub in range(NOSUB):
                        nc.tensor.transpose(
                            tp[:, osub * P:(osub + 1) * P],
                            wns[osub][:, dt * P:(dt + 1) * P], ident[:])
                    t = WTp.tile([P, OS], f32r, tag="WT")
                    nc.vector.tensor_copy(t[:], tp[:])
                    WTs.append(t)
            for ts in range(NTH * NTS):
                py = yps.tile([P, OS], f32, tag="yps")
                nc.tensor.matmul(py[:], ones[:1, :], bt[:1, :],
                                 start=True, stop=False)
                for d in range(ND):
                    nc.tensor.matmul(py[:], xhT[d][:, ts * P:(ts + 1) * P],
                                     WTs[d][:],
                                     start=False, stop=(d == ND - 1))
                yo = yout.tile([P, OS], f32, tag="yo")
                nc.scalar.copy(yo[:], py[:])
                nc.sync.dma_start(
                    y.ap()[ts * P:(ts + 1) * P,
                           os_ * OS:(os_ + 1) * OS], yo[:])

_CACHED_NC = None


def _get_nc():
    global _CACHED_NC
    if _CACHED_NC is None:
        _CACHED_NC = build_kernel()
    return _CACHED_NC


def _hadamard128():
    h = np.array([[1.0]], dtype=np.float32)
    while h.shape[0] < P:
        h = np.block([[h, h], [h, -h]])
    return h.astype(np.float32)


def kernel(x, W, b):
    x = np.asarray(x, dtype=np.float32)
    W = np.asarray(W, dtype=np.float32)
    b = np.asarray(b, dtype=np.float32)
    assert x.shape == (B, S, D) and W.shape == (O, D) and b.shape == (O,)

    nc = _get_nc()
    h128 = _hadamard128()
    consts = {
        "Hp": (h128 / 32.0).astype(np.float32),
        "Hn": (-h128 / 32.0).astype(np.float32),
        "Ident": np.eye(P, dtype=np.float32),
        "Ones": np.ones((1, P), np.float32),
    }
    xf = np.ascontiguousarray(x.reshape(B * S, D))
    in_maps = []
    for c in range(N_CORES):
        in_maps.append({
            "x": np.ascontiguousarray(xf[c * T_PER_CORE:(c + 1) * T_PER_CORE]),
            "W": W,
            "b": np.ascontiguousarray(b.reshape(1, O)),
            **consts,
        })
    res = run_bass_kernel_spmd(nc, in_maps, core_ids=list(range(N_CORES)))
    y = np.concatenate([res.results[c]["y"] for c in range(N_CORES)], axis=0)
    return y.reshape(B, S, O).astype(np.float32, copy=False)


# revision 7
# speedup vs baseline: 19640.4930x; 18907.3686x over previous
"""NoisyHadamardLinear Trainium2 kernel (self-contained).

y = blockwise_FHT_1024(x) @ W^T + b  for x [2, 4096, 4096], W [4096, 4096],
b [4096], on 8 NeuronCores, data-parallel over the 8192 tokens (1024/core).

Per-core pipeline (all matmuls fp32r on TensorE):
  phase H: PE-transpose x tiles -> xT chunks; apply H_128/32 as one matmul
           per 128-chunk with butterfly stage-1 folded into the PSUM
           accumulation (H_1024 = H_8 (x) H_128 Kronecker); butterfly
           stages 2-3 on VectorE -> xhT tiles [d, t] resident in SBUF.
  phase M: per 512-wide o-slab, PE-transpose W tiles on the fly -> WT;
           y[t, o] = sum_d xhT[d, t].T @ WT[d, o] accumulated over 32
           d-tiles in PSUM + bias rank-1 (ones x b) matmul; ACT evict; DMA.
"""
import numpy as np

import concourse.bacc as bacc
import concourse.mybir as mybir
import concourse.tile as tile
from concourse.bass_utils import run_bass_kernel_spmd

P = 128
f32r = mybir.dt.float32r
f32 = mybir.dt.float32

N_CORES = 8
B, S, D, O = 2, 4096, 4096, 4096
T_PER_CORE = (B * S) // N_CORES


def build_kernel(T=T_PER_CORE, D=D, O=O, OS=512, num_devices=N_CORES,
                 phases=('H', 'M')):
    NTH = 2 if T >= 1024 else 1            # t-halves
    TH = T // NTH                          # tokens per half
    NTS = TH // P                          # t-subtiles per half
    NBLK = D // 1024                       # hadamard blocks
    ND = D // P                            # d tiles
    NOS = O // OS                          # o-slabs
    NOSUB = OS // P                        # o-subtiles per slab

    nc = bacc.Bacc("TRN2", target_bir_lowering=False, debug=False,
                   num_devices=num_devices, dynamic_dma_scratch_size=2048)
    x = nc.dram_tensor("x", [T, D], f32r, kind="ExternalInput")
    W = nc.dram_tensor("W", [O, D], f32r, kind="ExternalInput")
    b = nc.dram_tensor("b", [1, O], f32r, kind="ExternalInput")
    Hp = nc.dram_tensor("Hp", [P, P], f32r, kind="ExternalInput")
    Hn = nc.dram_tensor("Hn", [P, P], f32r, kind="ExternalInput")
    Ident = nc.dram_tensor("Ident", [P, P], f32r, kind="ExternalInput")
    Ones = nc.dram_tensor("Ones", [1, P], f32r, kind="ExternalInput")
    y = nc.dram_tensor("y", [T, O], f32, kind="ExternalOutput")

    with tile.TileContext(nc) as tc:
        with tc.tile_pool(name="const", bufs=1) as cpool, \
             tc.tile_pool(name="xhT", bufs=ND) as xhTp:
            ident = cpool.tile([P, P], f32r)
            hp = cpool.tile([P, P], f32r)
            hn = cpool.tile([P, P], f32r)
            ones = cpool.tile([1, P], f32r)
            nc.sync.dma_start(ident[:], Ident.ap())
            nc.sync.dma_start(hp[:], Hp.ap())
            nc.sync.dma_start(hn[:], Hn.ap())
            nc.sync.dma_start(ones[:], Ones.ap())

            # persistent xhT tiles [128 d, T tokens]
            xhT = [xhTp.tile([P, T], f32r, tag="xhT", name=f"xhT{i}")
                   for i in range(ND)]

            if 'H' in phases:
                _phase_h(nc, tc, x, ident, hp, hn, xhT,
                         NTH, TH, NTS, NBLK)
            if 'M' in phases:
                _phase_m(nc, tc, W, b, ident, ones, xhT, y,
                         NTH, NTS, ND, NOS, NOSUB, OS, D)
    nc.compile()
    return nc


def _phase_h(nc, tc, x, ident, hp, hn, xhT, NTH, TH, NTS, NBLK):
    with tc.tile_pool(name="xnat", bufs=NTS + 1) as xnat, \
         tc.tile_pool(name="xTp", bufs=9) as xTp, \
         tc.tile_pool(name="bfp", bufs=20) as bfp, \
         tc.tile_pool(name="tps", bufs=4, space="PSUM") as tps, \
         tc.tile_pool(name="hps", bufs=4, space="PSUM") as hps:
        for th in range(NTH):
            for blk in range(NBLK):
                xns = []
                for ts in range(NTS):
                    xn = xnat.tile([P, 1024], f32r, tag="xn")
                    trow = (th * NTS + ts) * P
                    nc.sync.dma_start(
                        xn[:], x.ap()[trow:trow + P,
                                      blk * 1024:(blk + 1) * 1024])
                    xns.append(xn)
                # transpose x tiles -> xT chunks
                xTs = []
                for u in range(8):
                    tp = tps.tile([P, TH], f32r, tag="tps")
                    for ts in range(NTS):
                        nc.tensor.transpose(
                            tp[:, ts * P:(ts + 1) * P],
                            xns[ts][:, u * P:(u + 1) * P], ident[:])
                    t = xTp.tile([P, TH], f32r, tag="xT")
                    nc.scalar.copy(t[:], tp[:])
                    xTs.append(t)
                # H128/32 chunk matmuls with butterfly stage-1 folded into
                # PSUM accumulation: s_k = H(x_2k)+H(x_2k+1),
                # d_k = H(x_2k)-H(x_2k+1) (via -H on the second operand)
                cur = []
                for k in range(4):
                    for sign in range(2):
                        ph = hps.tile([P, TH], f32, tag="hps")
                        nc.tensor.matmul(ph[:], hp[:], xTs[2 * k][:],
                                         start=True, stop=False)
                        nc.tensor.matmul(ph[:],
                                         (hp if sign == 0 else hn)[:],
                                         xTs[2 * k + 1][:],
                                         start=False, stop=True)
                        z = bfp.tile([P, TH], f32r, tag="bf",
                                     name=f"z{th}_{blk}_{k}_{sign}")
                        nc.scalar.copy(z[:], ph[:])
                        cur.append(z)
                # H8 butterfly stages 2-3 on VectorE
                for s in range(1, 3):
                    stride = 1 << s
                    nxt = [bfp.tile([P, TH], f32r, tag="bf",
                                    name=f"bf{th}_{blk}_{s}_{v}")
                           if s < 2 else None
                           for v in range(8)]
                    for g in range(0, 8, 2 * stride):
                        for j in range(stride):
                            a = cur[g + j]
                            bb = cur[g + j + stride]
                            if s == 2:
                                oa = xhT[blk * 8 + g + j][
                                    :, th * TH:(th + 1) * TH]
                                ob = xhT[blk * 8 + g + j + stride][
                                    :, th * TH:(th + 1) * TH]
                            else:
                                oa = nxt[g + j][:]
                                ob = nxt[g + j + stride][:]
                            nc.vector.tensor_add(oa, a[:], bb[:])
                            nc.vector.tensor_sub(ob, a[:], bb[:])
                    cur = nxt


def _phase_m(nc, tc, W, b, ident, ones, xhT, y,
             NTH, NTS, ND, NOS, NOSUB, OS, D):
    NWCH = D // 512
    with tc.tile_pool(name="wnat", bufs=NOSUB + 1) as wnat, \
         tc.tile_pool(name="WTp", bufs=ND + 2) as WTp, \
         tc.tile_pool(name="bpool", bufs=2) as bpool, \
         tc.tile_pool(name="yout", bufs=2) as yout, \
         tc.tile_pool(name="tps", bufs=5, space="PSUM") as tps, \
         tc.tile_pool(name="yps", bufs=3, space="PSUM") as yps:
        for os_ in range(NOS):
            bt = bpool.tile([1, OS], f32r, tag="bt")
            nc.sync.dma_start(bt[:], b.ap()[:, os_ * OS:(os_ + 1) * OS])
            WTs = []
            for dch in range(NWCH):
                wns = []
                for osub in range(NOSUB):
                    wn = wnat.tile([P, 512], f32r, tag="wn")
                    orow = os_ * OS + osub * P
                    nc.sync.dma_start(
                        wn[:], W.ap()[orow:orow + P,
                                      dch * 512:(dch + 1) * 512])
                    wns.append(wn)
                for dt in range(4):
                    tp = tps.tile([P, OS], f32r, tag="tps")
                    for osub in range(NOSUB):
                        nc.tensor.transpose(
                            tp[:, osub * P:(osub + 1) * P],
                            wns[osub][:, dt * P:(dt + 1) * P], ident[:])
                    t = WTp.tile([P, OS], f32r, tag="WT")
                    if (dch * 4 + dt) % 2 == 0:
                        nc.vector.tensor_copy(t[:], tp[:])
                    else:
                        nc.scalar.copy(t[:], tp[:])
                    WTs.append(t)
            for ts in range(NTH * NTS):
                py = yps.tile([P, OS], f32, tag="yps")
                nc.tensor.matmul(py[:], ones[:1, :], bt[:1, :],
                                 start=True, stop=False)
                for d in range(ND):
                    nc.tensor.matmul(py[:], xhT[d][:, ts * P:(ts + 1) * P],
                                     WTs[d][:],
                                     start=False, stop=(d == ND - 1))
                yo = yout.tile([P, OS], f32, tag="yo")
                nc.scalar.copy(yo[:], py[:])
                nc.sync.dma_start(
                    y.ap()[ts * P:(ts + 1) * P,
                           os_ * OS:(os_ + 1) * OS], yo[:])

_CACHED_NC = None


def _get_nc():
    global _CACHED_NC
    if _CACHED_NC is None:
        _CACHED_NC = build_kernel()
    return _CACHED_NC


def _hadamard128():
    h = np.array([[1.0]], dtype=np.float32)
    while h.shape[0] < P:
        h = np.block([[h, h], [h, -h]])
    return h.astype(np.float32)


def kernel(x, W, b):
    x = np.asarray(x, dtype=np.float32)
    W = np.asarray(W, dtype=np.float32)
    b = np.asarray(b, dtype=np.float32)
    assert x.shape == (B, S, D) and W.shape == (O, D) and b.shape == (O,)

    nc = _get_nc()
    h128 = _hadamard128()
    consts = {
        "Hp": (h128 / 32.0).astype(np.float32),
        "Hn": (-h128 / 32.0).astype(np.float32),
        "Ident": np.eye(P, dtype=np.float32),
        "Ones": np.ones((1, P), np.float32),
    }
    xf = np.ascontiguousarray(x.reshape(B * S, D))
    in_maps = []
    for c in range(N_CORES):
        in_maps.append({
            "x": np.ascontiguousarray(xf[c * T_PER_CORE:(c + 1) * T_PER_CORE]),
            "W": W,
            "b": np.ascontiguousarray(b.reshape(1, O)),
            **consts,
        })
    res = run_bass_kernel_spmd(nc, in_maps, core_ids=list(range(N_CORES)))
    y = np.concatenate([res.results[c]["y"] for c in range(N_CORES)], axis=0)
    return y.reshape(B, S, O).astype(np.float32, copy=False)
